# revision 3
# baseline (speedup 1.0000x reference)
"""Trainium2 kernel for nn_AssocScan: out[t] = gates[t]*out[t-1] + inputs[t].

Full shapes: gates/inputs/out = (4, 8192, 1024) float32.

Strategy: the scan is independent per (b, d) lane; only the sequence
dim carries the recurrence. Shard d 8-ways across the NeuronCores
(128 d-lanes per core = exactly the 128 SBUF partitions), keep all of
b and the sequence on each core. Host-side, transpose to (d, b*n) so
each core's shard is a contiguous [128, 32768] block whose partition
rows are DMA-friendly contiguous sequences. No cross-core
communication is needed.

On-core: the recurrence runs along the free dim via the DVE
tensor_tensor_scan instruction (op0=mult, op1=add), chained via
initial = last column of the previous scan chunk, in-place over the x
tile. One [128, 8192] tile pair per (b)-chain, double-buffered; loads
are 2 MiB chunks (DMA efficiency drops below that), scans 2048 cols,
stores 2 MiB; the last chain tapers (4096/2048/1024/512/512) so the
kernel tail (last load -> last scan -> last store) stays ~2 us.
Loads stream on the SP HWDGE ring, stores on the ACT ring.

Execution: a jax shard_map over the 8 cores calling the compiled NEFF
(same path as concourse.bass2jax.run_bass_via_pjrt), but with inputs
pre-staged on device (device_put + block_until_ready) before launch —
PCIe uploads otherwise race the NEFF and steal HBM bandwidth from
already-running cores (~+20% on 1-3 random cores per run).
"""

import numpy as np

B, N, D = 4, 8192, 1024
NCORES = 8
P = D // NCORES        # 128 partitions per core

_RUNNER = None


def _build_nc():
    import concourse.bacc as bacc
    import concourse.mybir as mybir
    from concourse.tile import TileContext

    f32 = mybir.dt.float32
    nc = bacc.Bacc()
    g = nc.declare_dram_parameter("gates", [P, B * N], f32, isOutput=False)
    x = nc.declare_dram_parameter("inputs", [P, B * N], f32, isOutput=False)
    o = nc.declare_dram_parameter("out", [P, B * N], f32, isOutput=True)

    def spans(sizes):
        out, off = [], 0
        for s in sizes:
            out.append((off, off + s))
            off += s
        return out

    taper = [4096, 2048, 1024, 512, 512]
    body = dict(loads=[4096] * 2, scans=[2048] * 4, stores=[4096] * 2)
    tail = dict(loads=taper, scans=taper, stores=taper)
    chains = [body] * (B - 1) + [tail]

    with TileContext(nc) as tc:
        with tc.tile_pool(name="pool", bufs=2) as pool:
            for b in range(B):
                spec = chains[b]
                boff = b * N
                gt = pool.tile([P, N], f32, tag="g")
                xt = pool.tile([P, N], f32, tag="x")
                for s0, s1 in spans(spec["loads"]):
                    nc.sync.dma_start(out=gt[:, s0:s1],
                                      in_=g[:, boff + s0:boff + s1])
                    nc.sync.dma_start(out=xt[:, s0:s1],
                                      in_=x[:, boff + s0:boff + s1])
                prev = None
                store_spans = spans(spec["stores"])
                si = 0
                for s0, s1 in spans(spec["scans"]):
                    nc.vector.tensor_tensor_scan(
                        out=xt[:, s0:s1],
                        data0=gt[:, s0:s1],
                        data1=xt[:, s0:s1],
                        initial=0.0 if prev is None else prev,
                        op0=mybir.AluOpType.mult,
                        op1=mybir.AluOpType.add,
                    )
                    prev = xt[:, s1 - 1:s1]
                    while si < len(store_spans) and store_spans[si][1] <= s1:
                        t0, t1 = store_spans[si]
                        nc.scalar.dma_start(
                            out=o[:, boff + t0:boff + t1], in_=xt[:, t0:t1])
                        si += 1
                assert si == len(store_spans)
    nc.compile()
    return nc


class _Runner:
    """shard_map runner over 8 cores with device-resident inputs."""

    def __init__(self):
        import jax
        import concourse.mybir as mybir
        from jax.sharding import Mesh, PartitionSpec, NamedSharding
        from jax.experimental.shard_map import shard_map
        from concourse import bass2jax

        bass2jax.install_neuronx_cc_hook()
        nc = _build_nc()
        self.nc = nc

        partition_name = (
            nc.partition_id_tensor.name if nc.partition_id_tensor else None)
        in_names, out_names, out_avals, zero_shapes = [], [], [], []
        for alloc in nc.m.functions[0].allocations:
            if not isinstance(alloc, mybir.MemoryLocationSet):
                continue
            name = alloc.memorylocations[0].name
            if alloc.kind == "ExternalInput":
                if name != partition_name:
                    in_names.append(name)
            elif alloc.kind == "ExternalOutput":
                out_names.append(name)
                shape = tuple(alloc.tensor_shape)
                dtype = mybir.dt.np(alloc.dtype)
                out_avals.append(jax.core.ShapedArray(shape, dtype))
                zero_shapes.append((shape, dtype))
        n_params = len(in_names)
        all_names = in_names + out_names
        if partition_name is not None:
            all_names = all_names + [partition_name]

        def _body(*args):
            operands = list(args)
            if partition_name is not None:
                operands.append(bass2jax.partition_id_tensor())
            outs = bass2jax._bass_exec_p.bind(
                *operands,
                out_avals=tuple(out_avals),
                in_names=tuple(all_names),
                out_names=tuple(out_names),
                lowering_input_output_aliases=(),
                sim_require_finite=True,
                sim_require_nnan=True,
                nc=nc,
            )
            return tuple(outs)

        devices = jax.devices()[:NCORES]
        mesh = Mesh(np.asarray(devices), ("core",))
        donate = tuple(range(n_params, n_params + len(out_names)))
        self.fn = jax.jit(
            shard_map(
                _body, mesh=mesh,
                in_specs=(PartitionSpec("core"),) * (n_params + len(out_names)),
                out_specs=(PartitionSpec("core"),) * len(out_names),
                check_rep=False,
            ),
            donate_argnums=donate, keep_unused=True,
        )
        self.sharding = NamedSharding(mesh, PartitionSpec("core"))
        self.in_names = in_names
        self.out_names = out_names
        self.zero_shapes = zero_shapes
        self.jax = jax

    def run(self, in_map_concat):
        """in_map_concat: dict name -> (NCORES*shape0, ...) global array.
        Returns list of np output arrays (global, concat along axis 0)."""
        jax = self.jax
        args = [
            jax.device_put(in_map_concat[name], self.sharding)
            for name in self.in_names
        ]
        args += [
            jax.device_put(
                np.zeros((NCORES * s[0], *s[1:]), dt), self.sharding)
            for (s, dt) in self.zero_shapes
        ]
        jax.block_until_ready(args)
        outs = self.fn(*args)
        return [np.asarray(o) for o in outs]


def get_runner():
    global _RUNNER
    if _RUNNER is None:
        _RUNNER = _Runner()
    return _RUNNER


def kernel(gates, inputs):
    gates = np.asarray(gates, dtype=np.float32)
    inputs = np.asarray(inputs, dtype=np.float32)
    # (B, N, D) -> (D, B*N); row blocks of P are per-core shards, and
    # their axis-0 concat is exactly the global (NCORES*P, B*N) array.
    gt = np.ascontiguousarray(gates.reshape(B * N, D).T)
    xt = np.ascontiguousarray(inputs.reshape(B * N, D).T)
    r = get_runner()
    outs = r.run({"gates": gt, "inputs": xt})
    out_t = outs[r.out_names.index("out")]          # (D, B*N)
    return np.ascontiguousarray(out_t.T).reshape(B, N, D)


# revision 4
# speedup vs baseline: 1.0478x; 1.0478x over previous
"""Trainium2 kernel for nn_AssocScan: out[t] = gates[t]*out[t-1] + inputs[t].

Full shapes: gates/inputs/out = (4, 8192, 1024) float32.

Strategy: the scan is independent per (b, d) lane; only the sequence
dim carries the recurrence. Shard d 8-ways across the NeuronCores
(128 d-lanes per core = exactly the 128 SBUF partitions), keep all of
b and the sequence on each core. Host-side, transpose to (d, b*n) so
each core's shard is a contiguous [128, 32768] block whose partition
rows are DMA-friendly contiguous sequences. No cross-core
communication is needed.

On-core: the recurrence runs along the free dim via the DVE
tensor_tensor_scan instruction (op0=mult, op1=add), chained via
initial = last column of the previous scan chunk, in-place over the x
tile. One [128, 8192] tile pair per (b)-chain, double-buffered; loads
are 2 MiB chunks (DMA efficiency drops below that), scans 2048 cols,
stores 2 MiB; the last chain tapers (4096/2048/1024/512/512) so the
kernel tail (last load -> last scan -> last store) stays ~2 us.
Loads stream on the SP HWDGE ring, stores on the ACT ring.

Per-core: 32 MiB read + 16 MiB write, DMA-bound at ~420 GB/s ->
~120 us DMA + ~11 us framework prologue/epilogue.
"""

import numpy as np

B, N, D = 4, 8192, 1024
NCORES = 8
P = D // NCORES        # 128 partitions per core

_NC = None


def _build_nc():
    import concourse.bacc as bacc
    import concourse.mybir as mybir
    from concourse.tile import TileContext

    f32 = mybir.dt.float32
    nc = bacc.Bacc()
    g = nc.declare_dram_parameter("gates", [P, B * N], f32, isOutput=False)
    x = nc.declare_dram_parameter("inputs", [P, B * N], f32, isOutput=False)
    o = nc.declare_dram_parameter("out", [P, B * N], f32, isOutput=True)

    def spans(sizes):
        out, off = [], 0
        for s in sizes:
            out.append((off, off + s))
            off += s
        return out

    taper = [4096, 2048, 1024, 512, 512]
    body = dict(loads=[4096] * 2, scans=[2048] * 4, stores=[4096] * 2)
    tail = dict(loads=taper, scans=taper, stores=taper)
    chains = [body] * (B - 1) + [tail]

    with TileContext(nc) as tc:
        with tc.tile_pool(name="pool", bufs=2) as pool:
            for b in range(B):
                spec = chains[b]
                boff = b * N
                gt = pool.tile([P, N], f32, tag="g")
                xt = pool.tile([P, N], f32, tag="x")
                for s0, s1 in spans(spec["loads"]):
                    nc.sync.dma_start(out=gt[:, s0:s1],
                                      in_=g[:, boff + s0:boff + s1])
                    nc.sync.dma_start(out=xt[:, s0:s1],
                                      in_=x[:, boff + s0:boff + s1])
                prev = None
                store_spans = spans(spec["stores"])
                si = 0
                for s0, s1 in spans(spec["scans"]):
                    nc.vector.tensor_tensor_scan(
                        out=xt[:, s0:s1],
                        data0=gt[:, s0:s1],
                        data1=xt[:, s0:s1],
                        initial=0.0 if prev is None else prev,
                        op0=mybir.AluOpType.mult,
                        op1=mybir.AluOpType.add,
                    )
                    prev = xt[:, s1 - 1:s1]
                    while si < len(store_spans) and store_spans[si][1] <= s1:
                        t0, t1 = store_spans[si]
                        nc.scalar.dma_start(
                            out=o[:, boff + t0:boff + t1], in_=xt[:, t0:t1])
                        si += 1
                assert si == len(store_spans)
    nc.compile()
    return nc


def get_nc():
    global _NC
    if _NC is None:
        _NC = _build_nc()
    return _NC


def _shard(arr):
    # (B, N, D) -> (D, B*N) contiguous, then split into 8 row blocks
    t = np.ascontiguousarray(arr.reshape(B * N, D).T)
    return [t[i * P:(i + 1) * P] for i in range(NCORES)]


def kernel(gates, inputs):
    from concourse.bass_utils import run_bass_kernel_spmd

    gates = np.asarray(gates, dtype=np.float32)
    inputs = np.asarray(inputs, dtype=np.float32)
    g_shards = _shard(gates)
    x_shards = _shard(inputs)
    in_maps = [
        {"gates": g_shards[i], "inputs": x_shards[i]} for i in range(NCORES)
    ]
    res = run_bass_kernel_spmd(get_nc(), in_maps, core_ids=list(range(NCORES)))
    out_t = np.concatenate(
        [res.results[i]["out"] for i in range(NCORES)], axis=0)
    return np.ascontiguousarray(out_t.T).reshape(B, N, D)
